# revision 9
# baseline (speedup 1.0000x reference)
"""Trainium2 Bass kernel for batched displacement-operator construction.

Math: for each alpha_b,
    Da[b] = diag(u) @ (V @ diag(exp(-i r lam)) @ V.T) @ diag(v)
with u_i = w^i, v_j = (1/w)^j, w = i*alpha/|alpha|.  Since u_i*v_j = w^(i-j)
(|w| == 1 up to fp eps), the outer phase factor is a Toeplitz matrix whose
tiles are slices of a per-alpha [128, 1920] shifted-window table, precomputed
on the host.  On device per alpha: 2 real 1024^3 matmuls (cos and -sin parts,
float32r for full-rate fp32 on the PE), then a complex elementwise multiply
by the phase tiles (4 muls on DVE reading PSUM, 2 add/sub on GPSIMD).

Sharding: 16 alphas data-parallel over 8 cores (2 per core); evecs replicated.
"""

import sys

sys.path.insert(0, "/opt/trn_rl_repo")

import numpy as np

N = 1024
B = 16
NCORES = 8
APC = B // NCORES  # alphas per core
P = 128
KC = N // P  # contraction chunks
MC = N // P  # output row chunks
NT = 512  # matmul free-dim tile (fp32 PSUM bank)
NNT = N // NT  # output col chunks
WWIN = 1920  # phase-window free size
C0 = 896  # phase-window offset constant

_cache = {}


def _build_module():
    import concourse.bacc as bacc
    import concourse.mybir as mybir
    import concourse.tile as tile

    f32 = mybir.dt.float32
    f32r = mybir.dt.float32r
    Alu = mybir.AluOpType
    Act = mybir.ActivationFunctionType

    nc = bacc.Bacc(
        "TRN2",
        target_bir_lowering=False,
        debug=False,
        num_devices=NCORES,
    )

    vt_d = nc.dram_tensor("vt", [N, N], f32, kind="ExternalInput")
    esc_d = nc.dram_tensor("esc", [P, APC * 2 * KC], f32, kind="ExternalInput")
    ph_d = nc.dram_tensor("ph", [P, APC * 2 * WWIN], f32, kind="ExternalInput")
    outr_d = nc.dram_tensor("outr", [APC, N, N], f32, kind="ExternalOutput")
    outi_d = nc.dram_tensor("outi", [APC, N, N], f32, kind="ExternalOutput")

    with tile.TileContext(nc) as tc:
        with (
            tc.tile_pool(name="const", bufs=1) as cpool,
            tc.tile_pool(name="wts", bufs=1) as wpool,
            tc.tile_pool(name="work", bufs=3) as work,
            tc.tile_pool(name="outp", bufs=3) as outp,
            tc.tile_pool(name="psum", bufs=2, space="PSUM") as pp,
        ):
            vt = cpool.tile([P, KC, N], f32r)
            esc = cpool.tile([P, APC * 2 * KC], f32)
            ph = cpool.tile([P, APC * 2 * WWIN], f32)

            # The fp32r DRAM-input binding path crashes the exec unit, so
            # DMA fp32 and round to fp32r on-device (DVE cast producer).
            for kc in range(KC):
                tmp = work.tile([P, N], f32, tag="vtin")
                nc.sync.dma_start(tmp[:], vt_d[kc * P : (kc + 1) * P, :])
                nc.vector.tensor_copy(vt[:, kc, :], tmp[:])
            nc.sync.dma_start(esc[:], esc_d[:])
            nc.sync.dma_start(ph[:], ph_d[:])

            lc = wpool.tile([P, KC, N], f32r, tag="lc")
            ls = wpool.tile([P, KC, N], f32r, tag="ls")

            for a in range(APC):
                # Scale VT rows by er = cos(r*lam) and ei = -sin(r*lam)
                # (per-partition scalars) to form the matmul weights.
                for kc in range(KC):
                    col_er = a * 2 * KC + kc
                    col_ei = a * 2 * KC + KC + kc
                    nc.scalar.activation(
                        lc[:, kc, :], vt[:, kc, :], Act.Copy,
                        scale=esc[:, col_er : col_er + 1],
                    )
                    nc.scalar.activation(
                        ls[:, kc, :], vt[:, kc, :], Act.Copy,
                        scale=esc[:, col_ei : col_ei + 1],
                    )

                base_c = (a * 2) * WWIN
                base_s = (a * 2 + 1) * WWIN

                for m in range(MC):
                    pc0 = pp.tile([P, NT], f32, tag="pc0")
                    pc1 = pp.tile([P, NT], f32, tag="pc1")
                    ps0 = pp.tile([P, NT], f32, tag="ps0")
                    ps1 = pp.tile([P, NT], f32, tag="ps1")
                    for kc in range(KC):
                        lcap = lc[:, kc, m * P : (m + 1) * P]
                        lsap = ls[:, kc, m * P : (m + 1) * P]
                        vt0 = vt[:, kc, 0:NT]
                        vt1 = vt[:, kc, NT:N]
                        st = kc == 0
                        sp = kc == KC - 1
                        nc.tensor.matmul(pc0[:], lcap, vt0, start=st, stop=sp)
                        nc.tensor.matmul(pc1[:], lcap, vt1, start=st, stop=sp)
                        nc.tensor.matmul(ps0[:], lsap, vt0, start=st, stop=sp)
                        nc.tensor.matmul(ps1[:], lsap, vt1, start=st, stop=sp)
                    for n in range(NNT):
                        pc = pc0 if n == 0 else pc1
                        ps = ps0 if n == 0 else ps1
                        t0 = C0 - P * m + NT * n
                        pr = ph[:, base_c + t0 : base_c + t0 + NT]
                        pi = ph[:, base_s + t0 : base_s + t0 + NT]
                        m1 = work.tile([P, NT], f32, tag="m1")
                        m2 = work.tile([P, NT], f32, tag="m2")
                        m3 = work.tile([P, NT], f32, tag="m3")
                        m4 = work.tile([P, NT], f32, tag="m4")
                        nc.vector.tensor_tensor(m1[:], pc[:], pr, Alu.mult)
                        nc.vector.tensor_tensor(m2[:], ps[:], pi, Alu.mult)
                        nc.vector.tensor_tensor(m3[:], pc[:], pi, Alu.mult)
                        nc.vector.tensor_tensor(m4[:], ps[:], pr, Alu.mult)
                        dar = outp.tile([P, NT], f32, tag="dar")
                        dai = outp.tile([P, NT], f32, tag="dai")
                        nc.gpsimd.tensor_tensor(dar[:], m1[:], m2[:], Alu.subtract)
                        nc.gpsimd.tensor_tensor(dai[:], m3[:], m4[:], Alu.add)
                        nc.sync.dma_start(
                            outr_d[a, m * P : (m + 1) * P, n * NT : (n + 1) * NT],
                            dar[:],
                        )
                        nc.sync.dma_start(
                            outi_d[a, m * P : (m + 1) * P, n * NT : (n + 1) * NT],
                            dai[:],
                        )

    nc.compile()
    return nc


def _get_module():
    if "nc" not in _cache:
        _cache["nc"] = _build_module()
    return _cache["nc"]


def _host_precompute(alpha_real, alpha_imag, evals):
    """Per-alpha scalar tables, mirroring the reference's fp32 arithmetic."""
    ar = np.asarray(alpha_real, np.float32)
    ai = np.asarray(alpha_imag, np.float32)
    ev = np.asarray(evals, np.float32)

    esc_all = np.empty((B, 2, KC, P), np.float32)  # (b, er/ei, kc, p)
    ph_all = np.empty((B, 2, P, WWIN), np.float32)  # (b, re/im, p, w)

    prow = np.arange(P)[:, None]
    scol = np.arange(WWIN)[None, :]
    idx = (prow - scol) + C0 + (N - 1)  # into d-table of length 2N-1

    for b in range(B):
        alpha = np.complex64(complex(ar[b], ai[b]))
        r = np.float32(np.abs(alpha)) + np.float32(1e-10)
        eit = np.complex64(alpha / r)
        w = np.complex128(1j) * np.complex128(eit)

        t32 = (np.float32(r) * ev).astype(np.float32)
        t64 = t32.astype(np.float64)
        er = np.cos(t64).astype(np.float32)
        ei = (-np.sin(t64)).astype(np.float32)
        esc_all[b, 0] = er.reshape(KC, P)
        esc_all[b, 1] = ei.reshape(KC, P)

        d = np.arange(-(N - 1), N)
        ptab = w ** d  # complex128, |w|~1 so no overflow
        wc = ptab.real.astype(np.float32)
        ws = ptab.imag.astype(np.float32)
        ph_all[b, 0] = wc[idx]
        ph_all[b, 1] = ws[idx]

    return esc_all, ph_all


def kernel(alpha_real, alpha_imag, evals, evecs):
    from concourse import bass_utils

    nc = _get_module()

    evecs_f = np.ascontiguousarray(np.asarray(evecs, np.float32))
    vt_np = np.ascontiguousarray(evecs_f.T)
    esc_all, ph_all = _host_precompute(alpha_real, alpha_imag, evals)

    in_maps = []
    for c in range(NCORES):
        bs = [c * APC + a for a in range(APC)]
        # esc columns: per alpha [er cols | ei cols]; value at (p, col) with
        # col = a*2*KC + which*KC + kc  ->  esc_all[b, which, kc, p]
        esc = np.empty((P, APC * 2 * KC), np.float32)
        ph = np.empty((P, APC * 2 * WWIN), np.float32)
        for a, b in enumerate(bs):
            for which in range(2):
                cols = a * 2 * KC + which * KC
                esc[:, cols : cols + KC] = esc_all[b, which].T
                wbase = (a * 2 + which) * WWIN
                ph[:, wbase : wbase + WWIN] = ph_all[b, which]
        in_maps.append({"vt": vt_np, "esc": esc, "ph": ph})

    res = bass_utils.run_bass_kernel_spmd(
        nc, in_maps, core_ids=list(range(NCORES))
    )

    out = np.empty((B, N, N), np.complex64)
    for c in range(NCORES):
        outr = res.results[c]["outr"]
        outi = res.results[c]["outi"]
        for a in range(APC):
            b = c * APC + a
            out.real[b] = outr[a]
            out.imag[b] = outi[a]
    return out


# revision 41
# speedup vs baseline: 31835.4036x; 31835.4036x over previous
"""Trainium2 Bass kernel for batched displacement-operator construction.

Math: for each alpha_b,
    Da[b] = diag(u) @ (V @ diag(exp(-i r lam)) @ V.T) @ diag(v)
with u_i = w^i, v_j = (1/w)^j, w = i*alpha/|alpha|.  Since u_i*v_j = w^(i-j)
(|w| == 1 up to fp eps), the outer phase factor is a Toeplitz matrix whose
tiles are slices of a per-alpha [128, 1920] shifted-window table, precomputed
on the host.  On device per alpha: 2 real 1024^3 matmuls (cos and -sin parts,
float32r for full-rate fp32 on the PE), then a complex elementwise multiply
by the phase tiles (4 muls on DVE reading PSUM, 2 add/sub on GPSIMD).

Sharding: 16 alphas data-parallel over 8 cores (2 per core); evecs replicated.
"""

import sys

sys.path.insert(0, "/opt/trn_rl_repo")

import numpy as np

N = 1024
B = 16
NCORES = 8
APC = B // NCORES  # alphas per core
P = 128
KC = N // P  # contraction chunks
MC = N // P  # output row chunks
NT = 512  # matmul free-dim tile (fp32 PSUM bank)
NNT = N // NT  # output col chunks
WWIN = 1920  # phase-window free size
C0 = 896  # phase-window offset constant

_cache = {}


def _build_module(reps=1):
    import contextlib

    import concourse.bacc as bacc
    import concourse.mybir as mybir
    import concourse.tile as tile

    f32 = mybir.dt.float32
    f32r = mybir.dt.float32r
    Alu = mybir.AluOpType
    Act = mybir.ActivationFunctionType

    nc = bacc.Bacc(
        "TRN2",
        target_bir_lowering=False,
        debug=False,
        num_devices=NCORES,
    )

    vt_d = nc.dram_tensor("vt", [N, N], f32, kind="ExternalInput")
    esc_d = nc.dram_tensor("esc", [P, APC * 2 * KC], f32, kind="ExternalInput")
    ph_d = nc.dram_tensor("ph", [P, APC * 2 * WWIN], f32, kind="ExternalInput")
    outr_d = nc.dram_tensor("outr", [APC, N, N], f32, kind="ExternalOutput")
    outi_d = nc.dram_tensor("outi", [APC, N, N], f32, kind="ExternalOutput")

    with tile.TileContext(nc) as tc:
        with (
            tc.tile_pool(name="const", bufs=1) as cpool,
            tc.tile_pool(name="wts", bufs=1) as wpool,
            tc.tile_pool(name="work", bufs=3) as work,
            tc.tile_pool(name="outp", bufs=3) as outp,
            tc.tile_pool(name="psum", bufs=2, space="PSUM") as pp,
        ):
            esc = cpool.tile([P, APC * 2 * KC], f32)
            ph = cpool.tile([P, APC * 2 * WWIN], f32)

            # Per-chunk tiles so Tile tracks dependencies at chunk
            # granularity: the next alpha's weight scaling can overlap the
            # previous alpha's tail matmuls instead of waiting for them all.
            vt = [
                cpool.tile([P, N], f32r, tag=f"vt{kc}", name=f"vt{kc}")
                for kc in range(KC)
            ]
            lc = [
                wpool.tile([P, N], f32r, tag=f"lc{kc}", name=f"lc{kc}")
                for kc in range(KC)
            ]
            ls = [
                wpool.tile([P, N], f32r, tag=f"ls{kc}", name=f"ls{kc}")
                for kc in range(KC)
            ]

            # esc first (tiny, gates all weight scaling).  Split the vt
            # chunk loads between the HWDGE (sync) and SWDGE (gpsimd)
            # queues so they stream in parallel; ph goes last on SWDGE
            # since the phase tiles are first consumed much later.
            nc.gpsimd.dma_start(esc[:], esc_d[:])
            nc.gpsimd.dma_start(ph[:], ph_d[:])
            # The fp32r DRAM-input binding path crashes the exec unit, so
            # DMA fp32 and round to fp32r on-device (DVE cast producer).
            for kc in range(KC):
                tmp = work.tile([P, N], f32, tag="vtin")
                nc.sync.dma_start(tmp[:], vt_d[kc * P : (kc + 1) * P, :])
                nc.vector.tensor_copy(vt[kc][:], tmp[:])

            rep_ctx = (
                tc.For_i(0, reps, 1) if reps > 1 else contextlib.nullcontext()
            )
            with rep_ctx:
                _emit_body(nc, tc, vt, esc, ph, lc, ls, work, outp, pp,
                           outr_d, outi_d, mybir)

    nc.compile()
    return nc


def _emit_body(nc, tc, vt, esc, ph, lc, ls, work, outp, pp, outr_d, outi_d,
               mybir):
    f32 = mybir.dt.float32
    Alu = mybir.AluOpType
    Act = mybir.ActivationFunctionType
    if True:
            for a in range(APC):
                # Scale VT rows by er = cos(r*lam) and ei = -sin(r*lam)
                # (per-partition scalars) to form the matmul weights.
                for kc in range(KC):
                    col_er = a * 2 * KC + kc
                    col_ei = a * 2 * KC + KC + kc
                    # Split the scaling between ACT and DVE so neither is a
                    # serial bottleneck ahead of the matmuls.
                    nc.scalar.activation(
                        lc[kc][:], vt[kc][:], Act.Copy,
                        scale=esc[:, col_er : col_er + 1],
                    )
                    nc.vector.tensor_scalar_mul(
                        ls[kc][:], vt[kc][:], esc[:, col_ei : col_ei + 1]
                    )

                base_c = (a * 2) * WWIN
                base_s = (a * 2 + 1) * WWIN

                for m in range(MC):
                    pc0 = pp.tile([P, NT], f32, tag="pc0")
                    pc1 = pp.tile([P, NT], f32, tag="pc1")
                    ps0 = pp.tile([P, NT], f32, tag="ps0")
                    ps1 = pp.tile([P, NT], f32, tag="ps1")
                    # C = V diag(er) V^T is symmetric, so compute its
                    # transpose-free form with the UNSCALED vt block as the
                    # stationary operand (one fp32r weight load per kc
                    # serves all four moving streams — fp32r weight loads
                    # are expensive and not FWL-accelerated).
                    for kc in range(KC):
                        wap = vt[kc][:, m * P : (m + 1) * P]
                        st = kc == 0
                        sp = kc == KC - 1
                        nc.tensor.matmul(pc0[:], wap, lc[kc][:, 0:NT],
                                         start=st, stop=sp)
                        nc.tensor.matmul(pc1[:], wap, lc[kc][:, NT:N],
                                         start=st, stop=sp)
                        nc.tensor.matmul(ps0[:], wap, ls[kc][:, 0:NT],
                                         start=st, stop=sp)
                        nc.tensor.matmul(ps1[:], wap, ls[kc][:, NT:N],
                                         start=st, stop=sp)
                    for n in range(NNT):
                        pc = pc0 if n == 0 else pc1
                        ps = ps0 if n == 0 else ps1
                        t0 = C0 - P * m + NT * n
                        pr = ph[:, base_c + t0 : base_c + t0 + NT]
                        pi = ph[:, base_s + t0 : base_s + t0 + NT]
                        m1 = work.tile([P, NT], f32, tag="m1")
                        m2 = work.tile([P, NT], f32, tag="m2")
                        m3 = work.tile([P, NT], f32, tag="m3")
                        m4 = work.tile([P, NT], f32, tag="m4")
                        nc.vector.tensor_tensor(m1[:], pc[:], pr, Alu.mult)
                        nc.vector.tensor_tensor(m2[:], ps[:], pi, Alu.mult)
                        nc.vector.tensor_tensor(m3[:], pc[:], pi, Alu.mult)
                        nc.vector.tensor_tensor(m4[:], ps[:], pr, Alu.mult)
                        dar = outp.tile([P, NT], f32, tag="dar")
                        dai = outp.tile([P, NT], f32, tag="dai")
                        nc.gpsimd.tensor_tensor(dar[:], m1[:], m2[:], Alu.subtract)
                        nc.gpsimd.tensor_tensor(dai[:], m3[:], m4[:], Alu.add)
                        nc.sync.dma_start(
                            outr_d[a, m * P : (m + 1) * P, n * NT : (n + 1) * NT],
                            dar[:],
                        )
                        nc.sync.dma_start(
                            outi_d[a, m * P : (m + 1) * P, n * NT : (n + 1) * NT],
                            dai[:],
                        )


def _get_module():
    if "nc" not in _cache:
        _cache["nc"] = _build_module()
    return _cache["nc"]


def _host_precompute(alpha_real, alpha_imag, evals):
    """Per-alpha scalar tables, mirroring the reference's fp32 arithmetic."""
    ar = np.asarray(alpha_real, np.float32)
    ai = np.asarray(alpha_imag, np.float32)
    ev = np.asarray(evals, np.float32)

    esc_all = np.empty((B, 2, KC, P), np.float32)  # (b, er/ei, kc, p)
    ph_all = np.empty((B, 2, P, WWIN), np.float32)  # (b, re/im, p, w)

    prow = np.arange(P)[:, None]
    scol = np.arange(WWIN)[None, :]
    idx = (prow - scol) + C0 + (N - 1)  # into d-table of length 2N-1

    for b in range(B):
        alpha = np.complex64(complex(ar[b], ai[b]))
        r = np.float32(np.abs(alpha)) + np.float32(1e-10)
        eit = np.complex64(alpha / r)
        w = np.complex128(1j) * np.complex128(eit)

        t32 = (np.float32(r) * ev).astype(np.float32)
        t64 = t32.astype(np.float64)
        er = np.cos(t64).astype(np.float32)
        ei = (-np.sin(t64)).astype(np.float32)
        esc_all[b, 0] = er.reshape(KC, P)
        esc_all[b, 1] = ei.reshape(KC, P)

        d = np.arange(-(N - 1), N)
        ptab = w ** d  # complex128, |w|~1 so no overflow
        wc = ptab.real.astype(np.float32)
        ws = ptab.imag.astype(np.float32)
        ph_all[b, 0] = wc[idx]
        ph_all[b, 1] = ws[idx]

    return esc_all, ph_all


def kernel(alpha_real, alpha_imag, evals, evecs):
    from concourse import bass_utils

    nc = _get_module()

    evecs_f = np.ascontiguousarray(np.asarray(evecs, np.float32))
    vt_np = np.ascontiguousarray(evecs_f.T)
    esc_all, ph_all = _host_precompute(alpha_real, alpha_imag, evals)

    in_maps = []
    for c in range(NCORES):
        bs = [c * APC + a for a in range(APC)]
        # esc columns: per alpha [er cols | ei cols]; value at (p, col) with
        # col = a*2*KC + which*KC + kc  ->  esc_all[b, which, kc, p]
        esc = np.empty((P, APC * 2 * KC), np.float32)
        ph = np.empty((P, APC * 2 * WWIN), np.float32)
        for a, b in enumerate(bs):
            for which in range(2):
                cols = a * 2 * KC + which * KC
                esc[:, cols : cols + KC] = esc_all[b, which].T
                wbase = (a * 2 + which) * WWIN
                ph[:, wbase : wbase + WWIN] = ph_all[b, which]
        in_maps.append({"vt": vt_np, "esc": esc, "ph": ph})

    res = bass_utils.run_bass_kernel_spmd(
        nc, in_maps, core_ids=list(range(NCORES))
    )

    out = np.empty((B, N, N), np.complex64)
    for c in range(NCORES):
        outr = res.results[c]["outr"]
        outi = res.results[c]["outi"]
        for a in range(APC):
            b = c * APC + a
            out.real[b] = outr[a]
            out.imag[b] = outi[a]
    return out
